# revision 1
# baseline (speedup 1.0000x reference)
"""Trainium2 Bass kernel for nn_Cross_MultiAttention (8-head cross attention).

Sharding: one attention head per NeuronCore (8 heads / 8 cores).

Host folds the shared 1x1 input conv into each head's q/k/v projections
(Aq = wq_h @ w_in etc.).  Everything is padded to N=5120 tokens so all
loops are uniform; padded key columns are masked out, padded query
columns are sliced away on the host.

Per core:
  - project q/k/v for its head directly from (x+pos) / (context+pos).
  - attention scores are computed TRANSPOSED (keys on partitions, queries
    on the free dim) with K=32 contraction row-tiled 2x: two j-tiles'
    scores stream concurrently through disjoint 32-row bands of the PE
    array.
  - the pad/attention mask is applied ON THE PE: a matmul per score
    bank accumulates -240 * mask[j,i] into the score PSUM via a constant
    -240*I stationary with the fp8 mask tile as the moving operand.
  - softmax is max-free (|scores/16| < ~4); exp runs on the scalar engine
    straight from score PSUM into an fp8 probability store; the
    denominator comes from an appended ones-column in V.
  - P@V runs in fp8 with DoubleRow perf mode (two j-tiles contracted per
    pass).
  - the mask is host-retiled so each query stripe's mask slab loads as a
    single DMA with 20KB-contiguous per-partition runs.
Host divides each partial [256, N] by its denominator, sums the 8
partials, adds b_out, reshapes to [256, 50, 100].
"""

import numpy as np

import concourse.bacc as bacc
import concourse.tile as tile
import concourse.mybir as mybir
from concourse.bass_utils import run_bass_kernel_spmd

F32 = mybir.dt.float32
F32R = mybir.dt.float32r  # fp32 bits, full-rate PE streaming mode (rounded)
F16 = mybir.dt.float16
F8 = mybir.dt.float8e4
I32 = mybir.dt.int32
AF = mybir.ActivationFunctionType
ALU = mybir.AluOpType
DR = mybir.MatmulPerfMode.DoubleRow

EMB = 256
HEADS = 8
DEPTH = 32
IN_CH = 256
H, W = 50, 100
N_TOK = H * W          # 5000
N_PAD = 5120           # padded token count (40 j-tiles of 128, 10 stripes of 512)
WSZ = 512
NW = N_PAD // WSZ      # 10
NJ = N_PAD // 128      # 40
SCALE = EMB ** (-0.5)  # 1/16
NEG = 240.0            # mask weight; -240/16 = -15 inside the exp

# Schraudolph exp-from-bits constants: exp(s/16) ~= bitcast(int32(s*A + B))
A_SCH = (2.0 ** 23) / np.log(2.0) * SCALE
B_SCH = 127.0 * 2.0 ** 23 - 366392.0

DVE_EXP = False  # fraction of exp groups on the vector engine


def _use_dve_exp(w, g):
    return DVE_EXP and (w + g) % 4 == 3


def build_nc(num_devices=8):
    """Build the Bass module (same SPMD program for every core)."""
    nc = bacc.Bacc("TRN2", target_bir_lowering=False, debug=False,
                   num_devices=num_devices)

    D = DEPTH
    xp_d = nc.dram_tensor("xp", (IN_CH, N_PAD), F16, kind="ExternalInput").ap()
    cp_d = nc.dram_tensor("cp", (IN_CH, N_PAD), F16, kind="ExternalInput").ap()
    nm3_d = nc.dram_tensor("nm3", (128, NW, NJ, WSZ), F8,
                           kind="ExternalInput").ap()
    AqT_d = nc.dram_tensor("AqT", (IN_CH, 4 * D), F16, kind="ExternalInput").ap()
    cq_d = nc.dram_tensor("cq", (4 * D, 1), F32, kind="ExternalInput").ap()
    AkT_d = nc.dram_tensor("AkT", (IN_CH, 4 * D), F16, kind="ExternalInput").ap()
    ck_d = nc.dram_tensor("ck", (4 * D, 1), F32, kind="ExternalInput").ap()
    AvT_d = nc.dram_tensor("AvT", (IN_CH, D), F16, kind="ExternalInput").ap()
    cvb_d = nc.dram_tensor("cvb", (128, D), F32, kind="ExternalInput").ap()
    negI_d = nc.dram_tensor("negI", (128, 128), F8, kind="ExternalInput").ap()
    woT_d = nc.dram_tensor("woT", (D, EMB), F32R, kind="ExternalInput").ap()
    y_d = nc.dram_tensor("y", (EMB, N_PAD), F32, kind="ExternalOutput").ap()
    dn_d = nc.dram_tensor("dn", (1, N_PAD), F32, kind="ExternalOutput").ap()

    NT = N_PAD // 512  # projection tiles

    with tile.TileContext(nc) as tc:
        with (
            tc.tile_pool(name="persist", bufs=1) as persist,
            tc.tile_pool(name="consts", bufs=1) as consts,
        ):
            # ---- constants to SBUF ----
            AqT_sb = consts.tile([128, 2, 4 * D], F16)
            AkT_sb = consts.tile([128, 2, 4 * D], F16)
            AvT_sb = consts.tile([128, 2, D], F16)
            for ct in range(2):
                nc.sync.dma_start(AqT_sb[:, ct, :], AqT_d[ct * 128:(ct + 1) * 128, :])
                nc.sync.dma_start(AkT_sb[:, ct, :], AkT_d[ct * 128:(ct + 1) * 128, :])
                nc.sync.dma_start(AvT_sb[:, ct, :], AvT_d[ct * 128:(ct + 1) * 128, :])
            cq_sb = consts.tile([4 * D, 1], F32)
            nc.sync.dma_start(cq_sb[:, :], cq_d[:, :])
            ck_sb = consts.tile([4 * D, 1], F32)
            nc.sync.dma_start(ck_sb[:, :], ck_d[:, :])
            cvb_sb = consts.tile([128, D], F32)
            nc.sync.dma_start(cvb_sb[:, :], cvb_d[:, :])
            negI_sb = consts.tile([128, 128], F8)
            nc.sync.dma_start(negI_sb[:, :], negI_d[:, :])
            woT_sb = consts.tile([D, EMB], F32R)
            nc.sync.dma_start(woT_sb[:, :], woT_d[:, :])

            # ---- persistent activations ----
            qT = persist.tile([4 * D, N_PAD], F16)
            kT = persist.tile([4 * D, N_PAD], F16)
            v_sb = persist.tile([128, NJ, 48], F8)  # [j%128, jt, d | ones | pad]
            nc.any.memset(v_sb[:, :, :], 0.0)
            nc.any.memset(v_sb[:, :, D], 1.0)
            p_store = persist.tile([128, NJ, WSZ], F8)

            # ---- stage 1: project q/k/v straight from (x|context)+pos ----
            with (
                tc.tile_pool(name="proj_in", bufs=3) as proj_in,
                tc.tile_pool(name="qk_ps", bufs=2, space="PSUM") as qk_ps,
                tc.tile_pool(name="v_ps", bufs=2, space="PSUM") as v_ps,
            ):
                for t in range(NT):
                    n0 = t * 512
                    img_t = proj_in.tile([128, 2, 512], F16, name="img_t")
                    for ct in range(2):
                        nc.sync.dma_start(
                            img_t[:, ct, :],
                            xp_d[ct * 128:(ct + 1) * 128, n0:n0 + 512])
                    qps = qk_ps.tile([4 * D, 512], F32, name="qps")
                    for ct in range(2):
                        nc.tensor.matmul(qps[:, :], AqT_sb[:, ct, :],
                                         img_t[:, ct, :],
                                         start=(ct == 0), stop=(ct == 1))
                    nc.vector.tensor_scalar_add(qT[:, n0:n0 + 512], qps[:, :],
                                                cq_sb[:, :])

                for t in range(NT):
                    n0 = t * 512
                    img_t = proj_in.tile([128, 2, 512], F16, name="img_t")
                    for ct in range(2):
                        nc.sync.dma_start(
                            img_t[:, ct, :],
                            cp_d[ct * 128:(ct + 1) * 128, n0:n0 + 512])
                    kps = qk_ps.tile([4 * D, 512], F32, name="qps")
                    for ct in range(2):
                        nc.tensor.matmul(kps[:, :], AkT_sb[:, ct, :],
                                         img_t[:, ct, :],
                                         start=(ct == 0), stop=(ct == 1))
                    nc.vector.tensor_scalar_add(kT[:, n0:n0 + 512], kps[:, :],
                                                ck_sb[:, :])
                    # v projection for the 4 j-tiles inside this 512 stripe
                    for jj in range(4):
                        jt = 4 * t + jj
                        vps = v_ps.tile([128, D], F32, name="vps")
                        for ct in range(2):
                            nc.tensor.matmul(
                                vps[:, :],
                                img_t[:, ct, jj * 128:(jj + 1) * 128],
                                AvT_sb[:, ct, :],
                                start=(ct == 0), stop=(ct == 1))
                        nc.vector.tensor_add(v_sb[:, jt, 0:D], vps[:, :],
                                             cvb_sb[:, :])

            # ---- stage 2: pipelined attention + output projection ----
            with (
                tc.tile_pool(name="s_ps", bufs=2, space="PSUM") as s_pool,
                tc.tile_pool(name="av_ps", bufs=2, space="PSUM") as av_pool,
                tc.tile_pool(name="y_ps", bufs=2, space="PSUM") as y_ps,
                tc.tile_pool(name="slab", bufs=2) as slab_pool,
                tc.tile_pool(name="stage", bufs=2) as stage_pool,
                tc.tile_pool(name="out_sb", bufs=3) as out_pool,
            ):
                def epilogue(av_t, wp):
                    # unnormalized head output, denominator row, partial
                    # output projection for a finished stripe
                    i0 = wp * WSZ
                    unn = out_pool.tile([34, WSZ], F32R, name="unn")
                    nc.vector.tensor_copy(unn[:, :], av_t[:, :])
                    nc.sync.dma_start(dn_d[:, i0:i0 + WSZ],
                                      unn[D:D + 1, :].bitcast(F32))
                    for c2 in range(2):
                        yps = y_ps.tile([128, WSZ], F32, name="yps")
                        nc.tensor.matmul(
                            yps[:, :], woT_sb[:, c2 * 128:(c2 + 1) * 128],
                            unn[0:D, :], start=True, stop=True)
                        ysb = out_pool.tile([128, WSZ], F32, name="ysb")
                        nc.vector.tensor_copy(ysb[:, :], yps[:, :])
                        nc.sync.dma_start(
                            y_d[c2 * 128:(c2 + 1) * 128, i0:i0 + WSZ],
                            ysb[:, :])

                av = None
                pending = None  # (av, w) of the just-finished stripe
                for w in range(NW + 1):
                    if w < NW:
                        slab = slab_pool.tile([128, NJ, WSZ], F8, name="slab")
                        nc.sync.dma_start(slab[:, :, :], nm3_d[:, w, :, :])
                    if w >= 1:
                        av = av_pool.tile([34, WSZ], F32, name="av")
                    for g in range(NJ // 2):
                        if w >= 1:
                            # P@V' (DoubleRow fp8) for the PREVIOUS stripe
                            nc.tensor.matmul(
                                av[:, :],
                                v_sb[:, 2 * g:2 * g + 2, 0:34],
                                p_store[:, 2 * g:2 * g + 2, :],
                                start=(g == 0), stop=(g == NJ // 2 - 1),
                                perf_mode=DR)
                        if w < NW:
                            i0 = w * WSZ
                            s = s_pool.tile([128, 2, WSZ], F32, name="s")
                            for b in range(2):
                                jt = 2 * g + b
                                nc.tensor.matmul(
                                    s[:, b, :],
                                    kT[32 * b:32 * b + 32,
                                       jt * 128:(jt + 1) * 128],
                                    qT[32 * b:32 * b + 32, i0:i0 + WSZ],
                                    start=True, stop=False,
                                    skip_group_check=True)
                            for b in range(2):
                                jt = 2 * g + b
                                nc.tensor.matmul(
                                    s[:, b, :], negI_sb[:, :],
                                    slab[:, jt, :],
                                    start=False, stop=True,
                                    skip_group_check=True)
                            p_dst = p_store[:, 2 * g:2 * g + 2, :]
                            if _use_dve_exp(w, g):
                                st = stage_pool.tile([128, 2, WSZ], I32,
                                                     name="st")
                                nc.vector.tensor_scalar(
                                    st[:, :, :], s[:, :, :],
                                    float(A_SCH), float(B_SCH),
                                    op0=ALU.mult, op1=ALU.add)
                                nc.vector.tensor_copy(p_dst, st.bitcast(F32))
                            else:
                                nc.scalar.activation(
                                    p_dst, s[:, :, :],
                                    AF.Exp, scale=float(SCALE))
                        if g == 4 and pending is not None:
                            epilogue(*pending)
                            pending = None
                    if w >= 1:
                        pending = (av, w - 1)
                if pending is not None:
                    epilogue(*pending)

    nc.compile()
    return nc


def make_pos(row_embed, col_embed):
    """[EMB, H*W]; first half col embeds, second half row embeds."""
    d2 = row_embed.shape[1]
    pos = np.empty((EMB, H, W), np.float32)
    pos[:d2] = col_embed[:W].T[:, None, :]      # [d2, 1, W] -> broadcast H
    pos[d2:] = row_embed[:H].T[:, :, None]      # [d2, H, 1] -> broadcast W
    return pos.reshape(EMB, H * W)


def make_in_maps(x, context, pad_mask, row_embed, col_embed, w_in, b_in,
                 wq, bq, wk, bk, wv, bv, w_out, n_heads=HEADS):
    f8 = np.float64
    x = np.asarray(x, np.float32)
    context = np.asarray(context, np.float32)
    pad_mask = np.asarray(pad_mask)
    row_embed = np.asarray(row_embed, np.float32)
    col_embed = np.asarray(col_embed, np.float32)
    w_in = np.asarray(w_in, f8)
    b_in = np.asarray(b_in, f8)
    w_out = np.asarray(w_out, np.float32)
    wq, bq = np.asarray(wq, f8), np.asarray(bq, f8)
    wk, bk = np.asarray(wk, f8), np.asarray(bk, f8)
    wv, bv = np.asarray(wv, f8), np.asarray(bv, f8)

    pos = make_pos(row_embed, col_embed)
    xp = np.zeros((EMB, N_PAD), np.float16)
    xp[:, :N_TOK] = (x.reshape(EMB, N_TOK) + pos).astype(np.float16)
    cp = np.zeros((EMB, N_PAD), np.float16)
    cp[:, :N_TOK] = (context.reshape(EMB, N_TOK) + pos).astype(np.float16)

    import ml_dtypes
    # additive mask [j, i]: 1.0 where attention is masked (or j padded)
    am = np.zeros((N_PAD, N_PAD), np.float32)
    am[:N_TOK, :N_TOK] = pad_mask[0].T
    am[N_TOK:, :] = 1.0
    nm3 = np.ascontiguousarray(
        am.reshape(NJ, 128, NW, WSZ).transpose(1, 2, 0, 3)
    ).astype(ml_dtypes.float8_e4m3)
    negI = (-NEG * np.eye(128, dtype=np.float32)).astype(ml_dtypes.float8_e4m3)

    shared = {"xp": xp, "cp": cp, "nm3": nm3, "negI": negI}
    in_maps = []
    for h in range(n_heads):
        sl = slice(h * DEPTH, (h + 1) * DEPTH)
        Aq = wq[sl] @ w_in          # [D, IN_CH]
        cq = wq[sl] @ b_in + bq[sl]
        Ak = wk[sl] @ w_in
        ck = wk[sl] @ b_in + bk[sl]
        Av = wv[sl] @ w_in
        cv = wv[sl] @ b_in + bv[sl]
        f32c = lambda a: np.ascontiguousarray(a.astype(np.float32))
        in_maps.append(dict(
            shared,
            AqT=np.ascontiguousarray(np.tile(Aq.T, (1, 4)).astype(np.float16)),
            cq=f32c(np.tile(cq.reshape(DEPTH, 1), (4, 1))),
            AkT=np.ascontiguousarray(np.tile(Ak.T, (1, 4)).astype(np.float16)),
            ck=f32c(np.tile(ck.reshape(DEPTH, 1), (4, 1))),
            AvT=np.ascontiguousarray(Av.T.astype(np.float16)),
            cvb=f32c(np.broadcast_to(cv, (128, DEPTH))),
            woT=np.ascontiguousarray(w_out[:, sl].T),
        ))
    return in_maps


_CACHE = {}


def kernel(x, context, pad_mask, row_embed, col_embed, w_in, b_in,
           wq, bq, wk, bk, wv, bv, w_out, b_out):
    if "nc" not in _CACHE:
        _CACHE["nc"] = build_nc()
    nc = _CACHE["nc"]
    in_maps = make_in_maps(x, context, pad_mask, row_embed, col_embed,
                           w_in, b_in, wq, bq, wk, bk, wv, bv, w_out)
    res = run_bass_kernel_spmd(nc, in_maps, core_ids=list(range(HEADS)))
    y = np.zeros((EMB, N_TOK), np.float64)
    for c in range(HEADS):
        r = res.results[c]
        y += (r["y"][:, :N_TOK].astype(np.float64)
              / r["dn"][:, :N_TOK].astype(np.float64))
    y = (y + np.asarray(b_out, np.float64)[:, None]).astype(np.float32)
    return y.reshape(EMB, H, W)

